# revision 3
# baseline (speedup 1.0000x reference)
"""Causal self-attention (B=1, S=4096, D=768, H=12) on 8 Trainium2 NeuronCores.

v3: like v2 but the AllGather is split into two half-gathers by head half
(K rows / V cols for heads 0-5, then heads 6-11). gather0 fires after half
the projections; attention on head-pairs 0-2 runs while gather1 is in
flight, hiding most of the second collective's latency.

v2 of the sequence-parallel stride-8 interleave kernel:
  - all inputs staged to bf16 on host (halves input DMA + SBUF)
  - causally-trimmed score groups: group (t0, C) covers kv chunks
    t0..t0+C-1 with uniform width N = 512 - 16*t0 (query cols [16*t0, 512));
    the causal diagonal of chunk t0+ci lies in relative cols
    [16*ci, 16*ci+16), so a [128, C, 16*C] strided mask multiply suffices.
  - single combined AllGather of [kT | v] (one collective instead of two)
  - PSUM->SBUF copies for k/v/q staging on the ACT engine (idle during
    projections); exp remains the attention-phase bottleneck on ACT.
"""

import sys

sys.path.insert(0, "/opt/trn_rl_repo")

import numpy as np
import ml_dtypes

import concourse.bass as bass
import concourse.mybir as mybir
import concourse.tile as tile
from concourse import bacc
from concourse.bass_utils import run_bass_kernel_spmd

NCORES = 8
S, D, H, HD = 4096, 768, 12, 64
P = 128
DMC = D // P            # 6 chunks of the model dim
NQ = S // NCORES        # 512 local queries per core
SLOT = S // NCORES      # 512 kv rows per core
HP = H // 2             # 6 head pairs
NKV = S // P            # 32 kv chunks of 128
VW = 65                 # v columns per head incl. ones column
HVW = H * VW            # 780
DH = D // 2             # 384 k-rows per head half
HHW = 6 * VW            # 390 v cols per head half
KOFF = DH * SLOT        # kT elems in a half kv buffer (196608)
KV_ELEMS = KOFF + SLOT * HHW       # 396288 per half
F32 = mybir.dt.float32
BF16 = mybir.dt.bfloat16
SCALE = 1.0 / np.sqrt(HD)

# score groups: (t0, C, SW) covers kv chunks t0..t0+C-1, query cols
# [16*t0, 512), causally-trimmed width N = 512 - 16*t0 per chunk. SW is the
# PSUM slot stride: each matmul output [128, N] must stay inside one 512-f32
# PSUM bank, so chunks sit at offsets SW*ci with SW*ci mod 512 + N <= 512.
GROUPS = [(0, 2, 512), (2, 2, 512), (4, 2, 512), (6, 2, 512), (8, 2, 512),
          (10, 2, 512), (12, 2, 512), (14, 2, 512),
          (16, 4, 256), (20, 4, 256), (24, 8, 128)]
# mask pack offsets in the [P, 1344] masks tensor, per C
MOFF = {2: 0, 4: 2 * 32, 8: 2 * 32 + 4 * 64}
MASKW = 2 * 32 + 4 * 64 + 8 * 128  # 1344

_CACHE = {}


def _build_program(reps: int = 1, no_cc: bool = False, cc_nodep: bool = False):
    nc = bacc.Bacc("TRN2", target_bir_lowering=False, debug=False,
                   num_devices=NCORES)

    xqT = nc.dram_tensor("xqT", [D, NQ], BF16, kind="ExternalInput").ap()
    xkvT = nc.dram_tensor("xkvT", [D, SLOT], BF16, kind="ExternalInput").ap()
    wqT = nc.dram_tensor("wqT", [D, D], BF16, kind="ExternalInput").ap()
    wkT = nc.dram_tensor("wkT", [D, D], BF16, kind="ExternalInput").ap()
    wvT = nc.dram_tensor("wvT", [D, D], BF16, kind="ExternalInput").ap()
    wpT = nc.dram_tensor("wpT", [D, D], BF16, kind="ExternalInput").ap()
    masks = nc.dram_tensor("masks", [P, MASKW], BF16, kind="ExternalInput").ap()
    out = nc.dram_tensor("out", [NQ, D], F32, kind="ExternalOutput").ap()
    ext_ag = None
    if no_cc or cc_nodep:
        ext_ag = (nc.dram_tensor("kv0_ag_in", [NCORES, KV_ELEMS], BF16,
                                 kind="ExternalInput").ap(),
                  nc.dram_tensor("kv1_ag_in", [NCORES, KV_ELEMS], BF16,
                                 kind="ExternalInput").ap())

    with tile.TileContext(nc, num_cores=NCORES) as tc:
        for _ in range(reps):
            _kernel_body(tc, xqT, xkvT, wqT, wkT, wvT, wpT, masks, out,
                         ext_ag=ext_ag, cc_nodep=cc_nodep)
    nc.compile()
    return nc


def _kernel_body(tc, xqT, xkvT, wqT, wkT, wvT, wpT, masks, out, ext_ag=None,
                 cc_nodep=False):
    nc = tc.nc
    rg = [list(range(NCORES))]

    with (
        tc.tile_pool(name="const", bufs=1) as cpool,
        tc.tile_pool(name="dram", bufs=2, space="DRAM") as dram,
        tc.tile_pool(name="qpool", bufs=2) as qpool,
    ):
        # ---- persistent SBUF tensors -------------------------------------
        xqT_sb = cpool.tile([P, DMC, NQ], BF16, tag="xqT")
        xkvT_sb = cpool.tile([P, DMC, SLOT], BF16, tag="xkvT")
        wqT_sb = cpool.tile([P, DMC, D], BF16, tag="wqT")
        wkT_sb = cpool.tile([P, DMC, D], BF16, tag="wkT")
        wvT_sb = cpool.tile([P, DMC, D], BF16, tag="wvT")
        wpT_sb = cpool.tile([P, DMC, D], BF16, tag="wpT")
        masks_sb = cpool.tile([P, MASKW], BF16, tag="masks")
        qT_sb = qpool.tile([P, DMC, NQ], BF16, tag="qT")
        kstage = cpool.tile([P, DMC, SLOT], BF16, tag="kstage")
        vstage = cpool.tile([P, SLOT // P, H, VW], BF16, tag="vstage")
        yT_sb = cpool.tile([P, DMC, NQ], BF16, tag="yT")

        # ---- DRAM bounce + gathered buffers (kT and v, per head half) ----
        kv_dram = [dram.tile([KV_ELEMS], BF16, name=f"kv_dram{u}")
                   for u in range(2)]
        kv_ag = [dram.tile([NCORES, KV_ELEMS], BF16, addr_space="Shared",
                           name=f"kv_ag{u}")
                 for u in range(2)]

        # ---- load inputs (chunked so the first matmuls start early) ------
        xkvT_v = xkvT.rearrange("(c p) f -> p c f", p=P)
        wkT_v = wkT.rearrange("(c p) f -> p c f", p=P)
        for dmc in range(DMC):
            nc.sync.dma_start(out=wkT_sb[:, dmc, :], in_=wkT_v[:, dmc, :])
            nc.sync.dma_start(out=xkvT_sb[:, dmc, :], in_=xkvT_v[:, dmc, :])
        wvT_v = wvT.rearrange("(c p) f -> p c f", p=P)
        for dmc in range(DMC):
            nc.sync.dma_start(out=wvT_sb[:, dmc, :], in_=wvT_v[:, dmc, :])
        nc.sync.dma_start(out=xqT_sb, in_=xqT.rearrange("(c p) f -> p c f", p=P))
        nc.sync.dma_start(out=wqT_sb, in_=wqT.rearrange("(c p) f -> p c f", p=P))
        nc.sync.dma_start(out=wpT_sb, in_=wpT.rearrange("(c p) f -> p c f", p=P))
        nc.sync.dma_start(out=masks_sb, in_=masks)
        nc.gpsimd.memset(vstage[:, :, :, 64:65], 1.0)

        # per-half views of the bounce buffers
        kT_part = [kv_dram[u][0:KOFF].rearrange("(oc p c) -> p oc c",
                                                p=P, c=SLOT)
                   for u in range(2)]
        v_part = [kv_dram[u][KOFF:KV_ELEMS].rearrange("(sc p c) -> p sc c",
                                                      p=P, c=HHW)
                  for u in range(2)]

        # ---- projections + half-gathers ----------------------------------
        # half u: K rows [384u, 384(u+1)) (oc 3u..3u+2) and V cols for
        # heads 6u..6u+5 (og = u); gather u fires as soon as both land.
        with (
            tc.tile_pool(name="psum_k", bufs=2, space="PSUM") as pk,
            tc.tile_pool(name="psum_v", bufs=2, space="PSUM") as pv,
        ):
            for u in range(2):
                for oc3 in range(3):
                    oc = 3 * u + oc3
                    ps = pk.tile([P, SLOT], F32, tag="ps")
                    for dmc in range(DMC):
                        nc.tensor.matmul(
                            ps,
                            wkT_sb[:, dmc, P * oc:P * (oc + 1)],
                            xkvT_sb[:, dmc, :],
                            start=(dmc == 0), stop=(dmc == DMC - 1),
                        )
                    nc.vector.tensor_copy(kstage[:, oc, :], ps)
                    nc.sync.dma_start(out=kT_part[u][:, oc3, :],
                                      in_=kstage[:, oc, :])
                og = u
                for sc in range(SLOT // P):
                    ps = pv.tile([P, 384], F32, tag="ps")
                    for dmc in range(DMC):
                        nc.tensor.matmul(
                            ps,
                            xkvT_sb[:, dmc, P * sc:P * (sc + 1)],
                            wvT_sb[:, dmc, 384 * og:384 * (og + 1)],
                            start=(dmc == 0), stop=(dmc == DMC - 1),
                        )
                    for hh in range(6):
                        h = 6 * og + hh
                        nc.vector.tensor_copy(
                            vstage[:, sc, h, 0:64], ps[:, 64 * hh:64 * (hh + 1)]
                        )
                    nc.sync.dma_start(
                        out=v_part[u][:, sc, :],
                        in_=vstage.rearrange("p sc h w -> p sc (h w)")
                            [:, sc, VW * 6 * og:VW * 6 * (og + 1)])
                if ext_ag is None:
                    nc.gpsimd.collective_compute(
                        "AllGather", mybir.AluOpType.bypass, replica_groups=rg,
                        ins=[kv_dram[u].opt()], outs=[kv_ag[u].opt()],
                    )
        if ext_ag is not None:
            if cc_nodep:
                for u in range(2):
                    nc.gpsimd.collective_compute(
                        "AllGather", mybir.AluOpType.bypass, replica_groups=rg,
                        ins=[kv_dram[u].opt()], outs=[kv_ag[u].opt()],
                    )
            kv_ag = list(ext_ag)
        # per-half kT view [d, rank, col]; v view [rank, chunk, p, col]
        kT_ag_r = [kv_ag[u][:, 0:KOFF].rearrange("s (d c) -> d s c", c=SLOT)
                   for u in range(2)]
        vchunks = [kv_ag[u][:, KOFF:KV_ELEMS].rearrange(
                       "s (g p c) -> s g p c", p=P, c=HHW)
                   for u in range(2)]

        # ---- Q^T projection -> bf16 (overlaps with the collective) -------
        with tc.tile_pool(name="psum_q", bufs=2, space="PSUM") as pp:
            for oc in range(DMC):
                ps = pp.tile([P, NQ], F32, tag="ps")
                for dmc in range(DMC):
                    nc.tensor.matmul(
                        ps,
                        wqT_sb[:, dmc, P * oc:P * (oc + 1)],
                        xqT_sb[:, dmc, :],
                        start=(dmc == 0), stop=(dmc == DMC - 1),
                    )
                nc.vector.tensor_copy(qT_sb[:, oc, :], ps)

        # ---- attention ----------------------------------------------------
        with (
            tc.tile_pool(name="kv", bufs=8) as kvpool,
            tc.tile_pool(name="att", bufs=8) as apool,
            tc.tile_pool(name="ps_s", bufs=3, space="PSUM") as spool,
            tc.tile_pool(name="ps_y", bufs=2, space="PSUM") as ypool,
            tc.tile_pool(name="norm", bufs=4) as npool,
        ):
            for hp in range(HP):
                ytiles = [ypool.tile([VW, NQ], F32, tag="y",
                                     name=f"y_{hp}_{hh}") for hh in range(2)]
                u = hp // 3
                hpl = hp % 3          # head-pair index within the half
                for gi, (t0, C, SW) in enumerate(GROUPS):
                    N = NQ - 16 * t0
                    Q0 = 16 * t0
                    slot0, cb0 = t0 // 4, t0 % 4
                    # K chunk group: one DMA [128, C*128]
                    kbig = kvpool.tile([P, 8, P], BF16, tag="k",
                                       name=f"k_{hp}_{gi}")
                    if C <= 4:
                        kin = kT_ag_r[u][P * hpl:P * (hpl + 1), slot0,
                                         P * cb0:P * (cb0 + C)]
                    else:
                        kin = kT_ag_r[u][P * hpl:P * (hpl + 1), 6:8, :]
                    nc.sync.dma_start(
                        out=kbig[:, 0:C, :].rearrange("p a b -> p (a b)"),
                        in_=kin)
                    # V chunk group (incl. ones cols): one DMA [128, C*130]
                    vbig = kvpool.tile([P, 8, 2 * VW], BF16, tag="v",
                                       name=f"v_{hp}_{gi}")
                    nsl = (C + 3) // 4
                    for si in range(nsl):
                        cw = min(C - 4 * si, 4)
                        vin = vchunks[u][slot0 + si, cb0:cb0 + cw, :,
                                         2 * VW * hpl:2 * VW * (hpl + 1)]
                        nc.sync.dma_start(
                            out=vbig[:, 4 * si:4 * si + cw, :],
                            in_=vin.rearrange("g p c -> p g c"))
                    for hh in range(2):
                        h = 2 * hp + hh
                        oc, ro = h // 2, 64 * (h % 2)
                        st = spool.tile([P, 1024], F32, tag="s",
                                        name=f"s_{hp}_{gi}_{hh}")
                        at = apool.tile([P, 1024], BF16, tag="a",
                                        name=f"a_{hp}_{gi}_{hh}")
                        for ci in range(C):
                            nc.tensor.matmul(
                                st[:, SW * ci:SW * ci + N],
                                kbig[64 * hh:64 * (hh + 1), ci, :],
                                qT_sb[ro:ro + 64, oc, Q0:NQ],
                                start=True, stop=True,
                            )
                        # exp over the packed group (gap-free via 3D AP)
                        if SW == N:
                            src, dst = st[:, 0:C * N], at[:, 0:C * N]
                        else:
                            src = st.rearrange(
                                "p (g c) -> p g c", c=SW)[:, 0:C, 0:N]
                            dst = at.rearrange(
                                "p (g c) -> p g c", c=SW)[:, 0:C, 0:N]
                        nc.scalar.activation(
                            dst, src, mybir.ActivationFunctionType.Exp,
                            scale=float(SCALE),
                        )
                        # one strided mask multiply for the whole group:
                        # chunk ci's causal transition sits in relative query
                        # cols [16*ci, 16*ci+16); cols beyond 16*C are all 1.
                        W = 16 * C
                        av = at.rearrange("p (g c) -> p g c", c=SW)[:, 0:C, 0:W]
                        mv = masks_sb[:, MOFF[C]:MOFF[C] + C * W].rearrange(
                            "p (g c) -> p g c", c=W)
                        nc.vector.tensor_mul(av, av, mv)
                        # A @ [V | 1] accumulation
                        for ci in range(C):
                            t = t0 + ci
                            nc.tensor.matmul(
                                ytiles[hh][:, Q0:NQ],
                                vbig[:, ci, VW * hh:VW * (hh + 1)],
                                at[:, SW * ci:SW * ci + N],
                                start=(t == 0), stop=(t == NKV - 1),
                                skip_group_check=True,
                            )
                # normalize: y[0:64] * (1 / y[64]) -> yT_sb. The reciprocal
                # row is fed to the multiply through a stride-0 partition
                # broadcast AP so the Pool engine stays free for the
                # collectives (a gpsimd partition_broadcast here would queue
                # the next rep's AllGather behind end-of-attention work).
                for hh in range(2):
                    h = 2 * hp + hh
                    oc, ro = h // 2, 64 * (h % 2)
                    r = npool.tile([1, NQ], F32, tag="r", name=f"r_{hp}_{hh}")
                    nc.vector.reciprocal(r, ytiles[hh][64:65, :])
                    r_dram = dram.tile([1, NQ], F32, name=f"rd_{hp}_{hh}")
                    nc.sync.dma_start(out=r_dram, in_=r)
                    rbs = npool.tile([64, NQ], F32, tag="rb",
                                     name=f"rb_{hp}_{hh}")
                    nc.sync.dma_start(out=rbs,
                                      in_=r_dram.to_broadcast((64, NQ)))
                    nc.vector.tensor_tensor(
                        out=yT_sb[ro:ro + 64, oc, :],
                        in0=ytiles[hh][0:64, :], in1=rbs,
                        op=mybir.AluOpType.mult,
                    )

        # ---- output projection -------------------------------------------
        with (
            tc.tile_pool(name="psum_o", bufs=2, space="PSUM") as pp,
            tc.tile_pool(name="ostage", bufs=3) as opool,
        ):
            for sc in range(NQ // P):
                for og in range(2):
                    ps = pp.tile([P, 384], F32, tag="ps")
                    for ic in range(DMC):
                        nc.tensor.matmul(
                            ps,
                            yT_sb[:, ic, P * sc:P * (sc + 1)],
                            wpT_sb[:, ic, 384 * og:384 * (og + 1)],
                            start=(ic == 0), stop=(ic == DMC - 1),
                        )
                    ost = opool.tile([P, 384], F32, tag="o")
                    nc.vector.tensor_copy(ost, ps)
                    nc.sync.dma_start(
                        out=out[P * sc:P * (sc + 1), 384 * og:384 * (og + 1)],
                        in_=ost,
                    )


def _host_masks(j: int) -> np.ndarray:
    parts = []
    for C in (2, 4, 8):
        i = np.arange(P)[:, None, None]
        ci = np.arange(C)[None, :, None]
        p = np.arange(16 * C)[None, None, :]
        m = (P * ci + i <= 8 * p + j)
        parts.append(m.reshape(P, C * 16 * C))
    return np.concatenate(parts, axis=1).astype(ml_dtypes.bfloat16)


def kernel(x, Wq, Wk, Wv, Wp, **_):
    x = np.asarray(x, dtype=np.float32)
    B = x.shape[0]
    xf = x.reshape(S, D).astype(ml_dtypes.bfloat16)
    wqT = np.ascontiguousarray(np.asarray(Wq, np.float32).T).astype(
        ml_dtypes.bfloat16)
    wkT = np.ascontiguousarray(np.asarray(Wk, np.float32).T).astype(
        ml_dtypes.bfloat16)
    wvT = np.ascontiguousarray(np.asarray(Wv, np.float32).T).astype(
        ml_dtypes.bfloat16)
    wpT = np.ascontiguousarray(np.asarray(Wp, np.float32).T).astype(
        ml_dtypes.bfloat16)

    if "nc" not in _CACHE:
        _CACHE["nc"] = _build_program()
    nc = _CACHE["nc"]

    in_maps = []
    for j in range(NCORES):
        in_maps.append({
            "xqT": np.ascontiguousarray(xf[j::NCORES].T),
            "xkvT": np.ascontiguousarray(xf[SLOT * j:SLOT * (j + 1)].T),
            "wqT": wqT, "wkT": wkT, "wvT": wvT, "wpT": wpT,
            "masks": _host_masks(j),
        })

    res = run_bass_kernel_spmd(nc, in_maps, list(range(NCORES)))
    out = np.empty((S, D), np.float32)
    for j in range(NCORES):
        out[j::NCORES] = res.results[j]["out"]
    return out.reshape(B, S, D)


if __name__ == "__main__":
    rng = np.random.default_rng(0)
    x = rng.standard_normal((1, S, D), dtype=np.float32)
    ws = [rng.standard_normal((D, D), dtype=np.float32) / np.sqrt(D)
          for _ in range(4)]
    y = kernel(x, *ws)
    print("ran", y.shape, y.dtype)
